# revision 28
# baseline (speedup 1.0000x reference)
"""Dice-loss kernel for Trainium2 (Bass/Tile), 8-core data-parallel SPMD.

Strategy
--------
reference: pred = argmax_c(logits); for c in 1..4:
    inter_c = #{v : pred[v]==c and tgt[v]==c},  tsum_c = #{v : tgt[v]==c}
    dice_c = (2*inter_c + eps) / (inter_c + tsum_c + eps); loss = 1 - mean(dice)

The voxel axis (B*D*H*W = 7,077,888) is sharded 8 ways.  Host-side input
formatting (per-voxel, information-preserving maps only -- all 7M-voxel
reductions happen on device):
  - d_c = l_c - l0 (fp32 sub, fp16 store), c=1..4: argmax is per-voxel
    translation invariant, so pred==c iff d_c == max(d) and d_c >= 0.
  - one-hot labels t_c as fp8e4m3 planes (0.0/1.0 exact): feeds the PE
    confusion matmul directly.

Engine split (v4, balanced against the ~358 GB/s per-core HBM roofline):
  Sync:   ALL dma_starts (no compute waits ahead of DMA issue; the
          scalar-engine HWDGE ring caused head-of-line blocking when the
          per-tile relu sat in front of oh issues)
  Scalar: mz = relu(m') (the 0-clamp, off DVE) + final PSUM->SBUF copies
  DVE:    mab = pairwise max (TT @2x), m' = max(mab0, mab1) (TT @2x),
          ev = 4-plane is_ge vs broadcast mz (TT @2x)
  GpSimd: one-time ev ones-column memsets
  PE:     4 confusion matmuls per 128-chunk, 129 moving cols each: the
          ev layout interleaves a constant-1.0 column after every 128
          data cols, so PSUM col 128 accumulates colsums of the
          stationary one-hot = tsum_c FOR FREE (no separate ones-matmul;
          ones cols sit at col % 129 == 128 in every chunk layout, so a
          single strided memset per ev buffer survives all tiles).

All SBUF DMA dests are flat [p, 0:4*fd] so every transfer is one
contiguous run per partition (128 descriptors, 2-9KB packets).

Accuracy: fp16 d-plane ties give ~1.4e-4 relative error on the loss
(tolerance 2e-2).  Counts stay exact integers in fp32 accumulators.
"""

import sys
from contextlib import ExitStack

import numpy as np

for _p in ("/opt/trn_rl_repo", "/opt/pypackages"):
    if _p not in sys.path:
        sys.path.append(_p)

import ml_dtypes
import concourse.bacc as bacc
import concourse.bass as bass
import concourse.tile as tile
from concourse import mybir
from concourse.bass_utils import run_bass_kernel_spmd

# Problem shape (hardcoded per contract: kernel.py must be self-contained).
B, C, D, H, W = 2, 5, 96, 192, 192
N_CORES = 8
P = 128                      # SBUF partitions
NVOX = B * D * H * W         # 7,077,888 voxels
SHARD = NVOX // N_CORES      # 884,736 voxels per core
FTOT = SHARD // P            # 6,912 free elems per partition
TILES = [128, 128, 256, 512, 768, 1280, 1280, 1280, 1024, 256]
NT = len(TILES)
NCLS = C - 1                 # foreground classes 1..4
EPS = 1e-8
assert sum(TILES) == FTOT


def emit_dice_kernel(tc, dpl_ap, oh_ap, out_ap, p, tiles):
    """Emit the per-core dice partial-sums program into TileContext `tc`.

    dpl_ap: DRAM [p, 4*ftot] fp16  -- d-planes, tile-blocked: cols
            [4*base, 4*(base+fd)) hold tile i as [4, fd] row-major
    oh_ap:  DRAM [p, 4*ftot] fp8e4 -- one-hot planes, same blocking
    out_ap: DRAM [p, 4*129] f32 -- per class c: cols [129c, 129c+128)
            confusion block (host takes the trace = inter_c), col
            129c+128 = per-row tsum_c partials.
    """
    nc = tc.nc
    nt = len(tiles)
    fdmax = max(tiles)
    ftot = sum(tiles)
    nchmax = fdmax // 128
    evw = 4 * nchmax * 129   # ev tile width (interleaved ones cols)
    fp16 = mybir.dt.float16
    fp8 = mybir.dt.float8e4
    f32 = mybir.dt.float32
    Alu = mybir.AluOpType
    Act = mybir.ActivationFunctionType
    assert all(fd % 128 == 0 for fd in tiles)

    with ExitStack() as ctx:
        pool_d = ctx.enter_context(tc.tile_pool(name="d", bufs=6))
        pool_oh = ctx.enter_context(tc.tile_pool(name="oh", bufs=6))
        pool_mab = ctx.enter_context(tc.tile_pool(name="mab", bufs=3))
        pool_mz = ctx.enter_context(tc.tile_pool(name="mz", bufs=4))
        pool_ev = ctx.enter_context(tc.tile_pool(name="ev", bufs=3))
        pool_acc = ctx.enter_context(tc.tile_pool(name="acc", bufs=1))
        pool_ps = ctx.enter_context(tc.tile_pool(name="ps", bufs=1, space="PSUM"))

        outb = pool_acc.tile([p, 4 * 129], f32, tag="outb")
        cm = [
            pool_ps.tile([128, 129], f32, tag=f"cm{q}", name=f"cm{q}")
            for q in range(4)
        ]

        # The 3 physical ev buffers rotate; cols == 128 (mod 129) are the
        # constant-1.0 columns in EVERY tile's chunk layout (class bases
        # are multiples of 129), so one strided memset each lasts forever.
        ev_bufs = [
            pool_ev.tile([p, evw], fp16, tag="ev", name=f"ev{q}")
            for q in range(3)
        ]
        for evb in ev_bufs:
            ones_ap = bass.AP(
                tensor=evb.tensor,
                offset=evb.offset + 128,
                ap=[list(evb.ap[0]), [129, 4 * nchmax]],
            )
            nc.gpsimd.memset(ones_ap, 1.0)

        bases = []
        b0 = 0
        for fd in tiles:
            bases.append(b0)
            b0 += fd
        dvs, ohs = {}, {}

        def issue_dv(i):
            fd = tiles[i]
            dv = pool_d.tile([p, 4 * fdmax], fp16, tag="dv")
            src_d = bass.AP(
                tensor=dpl_ap.tensor,
                offset=4 * bases[i],
                ap=[[4 * ftot, p], [1, 4 * fd]],
            )
            nc.sync.dma_start(out=dv[:, 0 : 4 * fd], in_=src_d)
            dvs[i] = dv

        def issue_oh(i):
            fd = tiles[i]
            oh = pool_oh.tile([p, 4 * fdmax], fp8, tag="oh")
            src_o = bass.AP(
                tensor=oh_ap.tensor,
                offset=4 * bases[i],
                ap=[[4 * ftot, p], [1, 4 * fd]],
            )
            nc.sync.dma_start(out=oh[:, 0 : 4 * fd], in_=src_o)
            ohs[i] = oh

        # pre-issue a few tiles' transfers so the DMA engines ramp before
        # the first compute dependency lands.  dpl first, oh staggered
        # two tiles behind: the 16 SDMA engines round-robin across live
        # queues at packet granularity, so every concurrently-issued
        # transfer delays dpl0's completion (and with it the first DVE
        # op).  Tiles 0-2 are small for the same reason.
        PRE_DV = 4
        PRE_OH = 2
        for i in range(PRE_DV):
            issue_dv(i)
        for i in range(PRE_OH):
            issue_oh(i)

        def emit_tail(i, fd, dv, oh, mz):
            """ev compare + confusion matmuls for tile i.  Emitted one
            iteration late (software pipelining): ev_i waits on the ACT
            relu; putting mab_{i+1}/m'_{i+1} ahead of it in the DVE queue
            hides that latency instead of head-of-line stalling."""
            nch = fd // 128
            # e_c = (d_c >= mz) for all 4 classes in ONE op, written into
            # the interleaved [128 data | 1 one] chunk layout (4D APs;
            # innermost packed 128 keeps the 2x DVE mode).
            ev = pool_ev.tile([p, evw], fp16, tag="ev")
            ev_v = bass.AP(
                tensor=ev.tensor,
                offset=ev.offset,
                ap=[list(ev.ap[0]), [129 * nch, 4], [129, nch], [1, 128]],
            )
            dv_v = bass.AP(
                tensor=dv.tensor,
                offset=dv.offset,
                ap=[list(dv.ap[0]), [fd, 4], [128, nch], [1, 128]],
            )
            m_sl = mz[:, 0:fd]
            m_bc = bass.AP(
                tensor=m_sl.tensor,
                offset=m_sl.offset,
                ap=[list(m_sl.ap[0]), [0, 4], [128, nch], [1, 128]],
            )
            nc.vector.tensor_tensor(ev_v, dv_v, m_bc, Alu.is_ge)

            # PE: per 128-chunk, 4 confusion matmuls (fp8 stationary x
            # fp16 moving, 129 cols: 128 ev data + the 1.0 col -> PSUM
            # col 128 accumulates colsums of oh_c = tsum_c partials).
            # On the last tile, run class-major with the per-class final
            # PSUM->SBUF copy right after that class's stop matmul, so
            # staging overlaps the remaining classes' matmuls.
            first = i == 0
            last = i == nt - 1
            if not last:
                for k in range(nch):
                    o = k * 128
                    st = first and k == 0
                    for ci in range(4):
                        nc.tensor.matmul(
                            cm[ci],
                            oh[:, ci * fd + o : ci * fd + o + 128],
                            ev[:, ci * (129 * nch) + k * 129 : ci * (129 * nch) + k * 129 + 129],
                            start=st,
                            stop=False,
                        )
            else:
                for ci in range(4):
                    for k in range(nch):
                        o = k * 128
                        nc.tensor.matmul(
                            cm[ci],
                            oh[:, ci * fd + o : ci * fd + o + 128],
                            ev[:, ci * (129 * nch) + k * 129 : ci * (129 * nch) + k * 129 + 129],
                            start=False,
                            stop=k == nch - 1,
                        )
                    nc.scalar.activation(
                        outb[:, ci * 129 : (ci + 1) * 129], cm[ci], Act.Copy
                    )
                    # ship this class's block immediately so the output
                    # DMA overlaps the remaining classes' matmuls
                    nc.sync.dma_start(
                        out=out_ap[:, ci * 129 : (ci + 1) * 129],
                        in_=outb[:, ci * 129 : (ci + 1) * 129],
                    )

        DIST = 2  # tail pipeline distance (tiles)
        pend = []
        for i, fd in enumerate(tiles):
            if i + PRE_DV < nt:
                issue_dv(i + PRE_DV)
            if i + PRE_OH < nt:
                issue_oh(i + PRE_OH)
            dv = dvs.pop(i)
            oh = ohs.pop(i)

            # mab = pairwise max of the 4 d-planes (2 planes out)
            mab = pool_mab.tile([p, 2 * fdmax], fp16, tag="mab")
            nc.vector.tensor_tensor(
                mab[:, 0 : 2 * fd],
                dv[:, 0 : 2 * fd],
                dv[:, 2 * fd : 4 * fd],
                Alu.max,
            )
            # m' = max(mab0, mab1)
            mp = pool_mz.tile([p, fdmax], fp16, tag="mp")
            nc.vector.tensor_tensor(
                mp[:, 0:fd], mab[:, 0:fd], mab[:, fd : 2 * fd], Alu.max
            )
            # mz = max(m', 0) -- tensor_scalar runs @4x on DVE; keeping
            # the whole chain on one engine avoids cross-engine stalls
            # the Tile scheduler cannot model (its ACT cost model misses
            # the TRN2 SBUF-source errata, so it pins ev right behind an
            # ACT relu and the DVE head-of-line stalls ~1.3us per tile).
            mz = pool_mz.tile([p, fdmax], fp16, tag="mz")
            nc.vector.tensor_scalar(mz[:, 0:fd], mp[:, 0:fd], 0.0, None, Alu.max)

            pend.append((i, fd, dv, oh, mz))
            if len(pend) > DIST:
                emit_tail(*pend.pop(0))
        for t in pend:
            emit_tail(*t)

        # PSUM staging + per-class output DMAs were interleaved with the
        # last tile's matmuls above; nothing left to emit.


_PROGRAM_CACHE = {}


def build_program():
    key = (C, P, FTOT, tuple(TILES))
    if key in _PROGRAM_CACHE:
        return _PROGRAM_CACHE[key]
    nc = bacc.Bacc("TRN2", debug=False, target_bir_lowering=False)
    dpl = nc.dram_tensor(
        "dpl", [P, 4 * FTOT], mybir.dt.float16, kind="ExternalInput"
    )
    oh = nc.dram_tensor(
        "oh", [P, 4 * FTOT], mybir.dt.float8e4, kind="ExternalInput"
    )
    out1 = nc.dram_tensor(
        "out1", [P, 4 * 129], mybir.dt.float32, kind="ExternalOutput"
    )
    with tile.TileContext(nc) as tc:
        emit_dice_kernel(tc, dpl.ap(), oh.ap(), out1.ap(), P, TILES)
    nc.compile()
    _PROGRAM_CACHE[key] = nc
    return nc


def make_in_maps(input2, target1):
    lg = np.asarray(input2, dtype=np.float32)
    tg = np.asarray(target1)
    # d_c = l_c - l0 in fp32, stored fp16; one-hot labels as fp8 (exact)
    d16 = (lg[:, 1:C] - lg[:, 0:1]).astype(np.float16).reshape(B, NCLS, NVOX // B)
    tgf = tg.reshape(B, NVOX // B)
    shards_per_b = N_CORES // B
    s = (NVOX // B) // shards_per_b
    in_maps = []
    for core in range(N_CORES):
        b, q = divmod(core, shards_per_b)
        sl = slice(q * s, (q + 1) * s)
        dsh = d16[b, :, sl].reshape(NCLS, P, FTOT)
        tsh = tgf[b, sl].reshape(P, FTOT)
        dpl = np.empty((P, 4 * FTOT), dtype=np.float16)
        ohp = np.empty((P, 4 * FTOT), dtype=ml_dtypes.float8_e4m3fn)
        base = 0
        for fd in TILES:
            slt = slice(base, base + fd)
            blk = slice(4 * base, 4 * (base + fd))
            dpl[:, blk] = dsh[:, :, slt].transpose(1, 0, 2).reshape(P, 4 * fd)
            ohc = np.stack(
                [(tsh[:, slt] == c) for c in range(1, C)], axis=1
            )  # [P, 4, fd] bool
            ohp[:, blk] = ohc.reshape(P, 4 * fd).astype(ml_dtypes.float8_e4m3fn)
            base += fd
        in_maps.append({"dpl": dpl, "oh": ohp})
    return in_maps


def _finish(results):
    """Host-side reduction of per-core partials -> scalar loss (float32).

    out1 [P, 4*129]: per class block, cols [:128] confusion block
    (trace = inter_c), col 128 = per-row tsum_c partials.
    """
    inter = np.zeros(NCLS, dtype=np.float64)
    tsum = np.zeros(NCLS, dtype=np.float64)
    for r in results:
        o = r["out1"].astype(np.float64)
        for ci in range(NCLS):
            blk = o[:, ci * 129 : (ci + 1) * 129]
            inter[ci] += np.trace(blk[:, :128])
            tsum[ci] += blk[:, 128].sum()
    inter = inter.astype(np.float32)
    tsum = tsum.astype(np.float32)
    eps = np.float32(EPS)
    dice = (np.float32(2.0) * inter + eps) / (inter + tsum + eps)
    loss = np.float32(1.0) - np.mean(dice, dtype=np.float32)
    return np.array([loss], dtype=np.float32)


# test.py can set e.g. RUN_KWARGS.update(trace=True) to profile; the grader
# path leaves this empty.
RUN_KWARGS = {}
LAST_RESULT = None


def kernel(input2, target1):
    global LAST_RESULT
    nc = build_program()
    in_maps = make_in_maps(input2, target1)
    res = run_bass_kernel_spmd(nc, in_maps, core_ids=list(range(N_CORES)), **RUN_KWARGS)
    LAST_RESULT = res
    return _finish(res.results)


# revision 29
# speedup vs baseline: 1.0229x; 1.0229x over previous
"""Dice-loss kernel for Trainium2 (Bass/Tile), 8-core data-parallel SPMD.

Strategy
--------
reference: pred = argmax_c(logits); for c in 1..4:
    inter_c = #{v : pred[v]==c and tgt[v]==c},  tsum_c = #{v : tgt[v]==c}
    dice_c = (2*inter_c + eps) / (inter_c + tsum_c + eps); loss = 1 - mean(dice)

The voxel axis (B*D*H*W = 7,077,888) is sharded 8 ways.  Host-side input
formatting (per-voxel, information-preserving maps only -- all 7M-voxel
reductions happen on device):
  - d_c = l_c - l0 (fp32 sub, fp16 store), c=1..4: argmax is per-voxel
    translation invariant, so pred==c iff d_c == max(d) and d_c >= 0.
  - one-hot labels t_c as fp8e4m3 planes (0.0/1.0 exact): feeds the PE
    confusion matmul directly.

Engine split (v4, balanced against the ~358 GB/s per-core HBM roofline):
  Sync:   ALL dma_starts (no compute waits ahead of DMA issue; the
          scalar-engine HWDGE ring caused head-of-line blocking when the
          per-tile relu sat in front of oh issues)
  Scalar: mz = relu(m') (the 0-clamp, off DVE) + final PSUM->SBUF copies
  DVE:    mab = pairwise max (TT @2x), m' = max(mab0, mab1) (TT @2x),
          ev = 4-plane is_ge vs broadcast mz (TT @2x)
  GpSimd: one-time ev ones-column memsets
  PE:     4 confusion matmuls per 128-chunk, 129 moving cols each: the
          ev layout interleaves a constant-1.0 column after every 128
          data cols, so PSUM col 128 accumulates colsums of the
          stationary one-hot = tsum_c FOR FREE (no separate ones-matmul;
          ones cols sit at col % 129 == 128 in every chunk layout, so a
          single strided memset per ev buffer survives all tiles).

All SBUF DMA dests are flat [p, 0:4*fd] so every transfer is one
contiguous run per partition (128 descriptors, 2-9KB packets).

Accuracy: fp16 d-plane ties give ~1.4e-4 relative error on the loss
(tolerance 2e-2).  Counts stay exact integers in fp32 accumulators.
"""

import sys
from contextlib import ExitStack

import numpy as np

for _p in ("/opt/trn_rl_repo", "/opt/pypackages"):
    if _p not in sys.path:
        sys.path.append(_p)

import ml_dtypes
import concourse.bacc as bacc
import concourse.bass as bass
import concourse.tile as tile
from concourse import mybir
from concourse.bass_utils import run_bass_kernel_spmd

# Problem shape (hardcoded per contract: kernel.py must be self-contained).
B, C, D, H, W = 2, 5, 96, 192, 192
N_CORES = 8
P = 128                      # SBUF partitions
NVOX = B * D * H * W         # 7,077,888 voxels
SHARD = NVOX // N_CORES      # 884,736 voxels per core
FTOT = SHARD // P            # 6,912 free elems per partition
TILES = [128, 256, 512, 896, 1280, 1280, 1280, 1024, 256]
NT = len(TILES)
NCLS = C - 1                 # foreground classes 1..4
EPS = 1e-8
assert sum(TILES) == FTOT


def emit_dice_kernel(tc, dpl_ap, oh_ap, out_ap, p, tiles):
    """Emit the per-core dice partial-sums program into TileContext `tc`.

    dpl_ap: DRAM [p, 4*ftot] fp16  -- d-planes, tile-blocked: cols
            [4*base, 4*(base+fd)) hold tile i as [4, fd] row-major
    oh_ap:  DRAM [p, 4*ftot] fp8e4 -- one-hot planes, same blocking
    out_ap: DRAM [p, 4*129] f32 -- per class c: cols [129c, 129c+128)
            confusion block (host takes the trace = inter_c), col
            129c+128 = per-row tsum_c partials.
    """
    nc = tc.nc
    nt = len(tiles)
    fdmax = max(tiles)
    ftot = sum(tiles)
    nchmax = fdmax // 128
    evw = 4 * nchmax * 129   # ev tile width (interleaved ones cols)
    fp16 = mybir.dt.float16
    fp8 = mybir.dt.float8e4
    f32 = mybir.dt.float32
    Alu = mybir.AluOpType
    Act = mybir.ActivationFunctionType
    assert all(fd % 128 == 0 for fd in tiles)

    with ExitStack() as ctx:
        pool_d = ctx.enter_context(tc.tile_pool(name="d", bufs=6))
        pool_oh = ctx.enter_context(tc.tile_pool(name="oh", bufs=6))
        pool_mab = ctx.enter_context(tc.tile_pool(name="mab", bufs=3))
        pool_mz = ctx.enter_context(tc.tile_pool(name="mz", bufs=4))
        pool_ev = ctx.enter_context(tc.tile_pool(name="ev", bufs=3))
        pool_acc = ctx.enter_context(tc.tile_pool(name="acc", bufs=1))
        pool_ps = ctx.enter_context(tc.tile_pool(name="ps", bufs=1, space="PSUM"))

        outb = pool_acc.tile([p, 4 * 129], f32, tag="outb")
        cm = [
            pool_ps.tile([128, 129], f32, tag=f"cm{q}", name=f"cm{q}")
            for q in range(4)
        ]

        # The 3 physical ev buffers rotate; cols == 128 (mod 129) are the
        # constant-1.0 columns in EVERY tile's chunk layout (class bases
        # are multiples of 129), so one strided memset each lasts forever.
        ev_bufs = [
            pool_ev.tile([p, evw], fp16, tag="ev", name=f"ev{q}")
            for q in range(3)
        ]
        for evb in ev_bufs:
            ones_ap = bass.AP(
                tensor=evb.tensor,
                offset=evb.offset + 128,
                ap=[list(evb.ap[0]), [129, 4 * nchmax]],
            )
            nc.gpsimd.memset(ones_ap, 1.0)

        bases = []
        b0 = 0
        for fd in tiles:
            bases.append(b0)
            b0 += fd
        dvs, ohs = {}, {}

        def issue_dv(i):
            fd = tiles[i]
            dv = pool_d.tile([p, 4 * fdmax], fp16, tag="dv")
            src_d = bass.AP(
                tensor=dpl_ap.tensor,
                offset=4 * bases[i],
                ap=[[4 * ftot, p], [1, 4 * fd]],
            )
            nc.sync.dma_start(out=dv[:, 0 : 4 * fd], in_=src_d)
            dvs[i] = dv

        def issue_oh(i):
            fd = tiles[i]
            oh = pool_oh.tile([p, 4 * fdmax], fp8, tag="oh")
            src_o = bass.AP(
                tensor=oh_ap.tensor,
                offset=4 * bases[i],
                ap=[[4 * ftot, p], [1, 4 * fd]],
            )
            nc.sync.dma_start(out=oh[:, 0 : 4 * fd], in_=src_o)
            ohs[i] = oh

        # pre-issue a few tiles' transfers so the DMA engines ramp before
        # the first compute dependency lands.  dpl first, oh staggered
        # two tiles behind: the 16 SDMA engines round-robin across live
        # queues at packet granularity, so every concurrently-issued
        # transfer delays dpl0's completion (and with it the first DVE
        # op).  Tiles 0-2 are small for the same reason.
        PRE_DV = 4
        PRE_OH = 2
        for i in range(PRE_DV):
            issue_dv(i)
        for i in range(PRE_OH):
            issue_oh(i)

        def emit_tail(i, fd, dv, oh, mz):
            """ev compare + confusion matmuls for tile i.  Emitted one
            iteration late (software pipelining): ev_i waits on the ACT
            relu; putting mab_{i+1}/m'_{i+1} ahead of it in the DVE queue
            hides that latency instead of head-of-line stalling."""
            nch = fd // 128
            # e_c = (d_c >= mz) for all 4 classes in ONE op, written into
            # the interleaved [128 data | 1 one] chunk layout (4D APs;
            # innermost packed 128 keeps the 2x DVE mode).
            ev = pool_ev.tile([p, evw], fp16, tag="ev")
            ev_v = bass.AP(
                tensor=ev.tensor,
                offset=ev.offset,
                ap=[list(ev.ap[0]), [129 * nch, 4], [129, nch], [1, 128]],
            )
            dv_v = bass.AP(
                tensor=dv.tensor,
                offset=dv.offset,
                ap=[list(dv.ap[0]), [fd, 4], [128, nch], [1, 128]],
            )
            m_sl = mz[:, 0:fd]
            m_bc = bass.AP(
                tensor=m_sl.tensor,
                offset=m_sl.offset,
                ap=[list(m_sl.ap[0]), [0, 4], [128, nch], [1, 128]],
            )
            nc.vector.tensor_tensor(ev_v, dv_v, m_bc, Alu.is_ge)

            # PE: per 128-chunk, 4 confusion matmuls (fp8 stationary x
            # fp16 moving, 129 cols: 128 ev data + the 1.0 col -> PSUM
            # col 128 accumulates colsums of oh_c = tsum_c partials).
            # On the last tile, run class-major with the per-class final
            # PSUM->SBUF copy right after that class's stop matmul, so
            # staging overlaps the remaining classes' matmuls.
            first = i == 0
            last = i == nt - 1
            if not last:
                for k in range(nch):
                    o = k * 128
                    st = first and k == 0
                    for ci in range(4):
                        nc.tensor.matmul(
                            cm[ci],
                            oh[:, ci * fd + o : ci * fd + o + 128],
                            ev[:, ci * (129 * nch) + k * 129 : ci * (129 * nch) + k * 129 + 129],
                            start=st,
                            stop=False,
                        )
            else:
                for ci in range(4):
                    for k in range(nch):
                        o = k * 128
                        nc.tensor.matmul(
                            cm[ci],
                            oh[:, ci * fd + o : ci * fd + o + 128],
                            ev[:, ci * (129 * nch) + k * 129 : ci * (129 * nch) + k * 129 + 129],
                            start=False,
                            stop=k == nch - 1,
                        )
                    nc.scalar.activation(
                        outb[:, ci * 129 : (ci + 1) * 129], cm[ci], Act.Copy
                    )
                    # ship this class's block immediately so the output
                    # DMA overlaps the remaining classes' matmuls
                    nc.sync.dma_start(
                        out=out_ap[:, ci * 129 : (ci + 1) * 129],
                        in_=outb[:, ci * 129 : (ci + 1) * 129],
                    )

        DIST = 2  # tail pipeline distance (tiles)
        pend = []
        for i, fd in enumerate(tiles):
            if i + PRE_DV < nt:
                issue_dv(i + PRE_DV)
            if i + PRE_OH < nt:
                issue_oh(i + PRE_OH)
            dv = dvs.pop(i)
            oh = ohs.pop(i)

            # mab = pairwise max of the 4 d-planes (2 planes out)
            mab = pool_mab.tile([p, 2 * fdmax], fp16, tag="mab")
            nc.vector.tensor_tensor(
                mab[:, 0 : 2 * fd],
                dv[:, 0 : 2 * fd],
                dv[:, 2 * fd : 4 * fd],
                Alu.max,
            )
            # m' = max(mab0, mab1)
            mp = pool_mz.tile([p, fdmax], fp16, tag="mp")
            nc.vector.tensor_tensor(
                mp[:, 0:fd], mab[:, 0:fd], mab[:, fd : 2 * fd], Alu.max
            )
            # mz = max(m', 0) -- tensor_scalar runs @4x on DVE; keeping
            # the whole chain on one engine avoids cross-engine stalls
            # the Tile scheduler cannot model (its ACT cost model misses
            # the TRN2 SBUF-source errata, so it pins ev right behind an
            # ACT relu and the DVE head-of-line stalls ~1.3us per tile).
            mz = pool_mz.tile([p, fdmax], fp16, tag="mz")
            nc.vector.tensor_scalar(mz[:, 0:fd], mp[:, 0:fd], 0.0, None, Alu.max)

            pend.append((i, fd, dv, oh, mz))
            if len(pend) > DIST:
                emit_tail(*pend.pop(0))
        for t in pend:
            emit_tail(*t)

        # PSUM staging + per-class output DMAs were interleaved with the
        # last tile's matmuls above; nothing left to emit.


_PROGRAM_CACHE = {}


def build_program():
    key = (C, P, FTOT, tuple(TILES))
    if key in _PROGRAM_CACHE:
        return _PROGRAM_CACHE[key]
    nc = bacc.Bacc("TRN2", debug=False, target_bir_lowering=False)
    dpl = nc.dram_tensor(
        "dpl", [P, 4 * FTOT], mybir.dt.float16, kind="ExternalInput"
    )
    oh = nc.dram_tensor(
        "oh", [P, 4 * FTOT], mybir.dt.float8e4, kind="ExternalInput"
    )
    out1 = nc.dram_tensor(
        "out1", [P, 4 * 129], mybir.dt.float32, kind="ExternalOutput"
    )
    with tile.TileContext(nc) as tc:
        emit_dice_kernel(tc, dpl.ap(), oh.ap(), out1.ap(), P, TILES)
    nc.compile()
    _PROGRAM_CACHE[key] = nc
    return nc


def make_in_maps(input2, target1):
    lg = np.asarray(input2, dtype=np.float32)
    tg = np.asarray(target1)
    # d_c = l_c - l0 in fp32, stored fp16; one-hot labels as fp8 (exact)
    d16 = (lg[:, 1:C] - lg[:, 0:1]).astype(np.float16).reshape(B, NCLS, NVOX // B)
    tgf = tg.reshape(B, NVOX // B)
    shards_per_b = N_CORES // B
    s = (NVOX // B) // shards_per_b
    in_maps = []
    for core in range(N_CORES):
        b, q = divmod(core, shards_per_b)
        sl = slice(q * s, (q + 1) * s)
        dsh = d16[b, :, sl].reshape(NCLS, P, FTOT)
        tsh = tgf[b, sl].reshape(P, FTOT)
        dpl = np.empty((P, 4 * FTOT), dtype=np.float16)
        ohp = np.empty((P, 4 * FTOT), dtype=ml_dtypes.float8_e4m3fn)
        base = 0
        for fd in TILES:
            slt = slice(base, base + fd)
            blk = slice(4 * base, 4 * (base + fd))
            dpl[:, blk] = dsh[:, :, slt].transpose(1, 0, 2).reshape(P, 4 * fd)
            ohc = np.stack(
                [(tsh[:, slt] == c) for c in range(1, C)], axis=1
            )  # [P, 4, fd] bool
            ohp[:, blk] = ohc.reshape(P, 4 * fd).astype(ml_dtypes.float8_e4m3fn)
            base += fd
        in_maps.append({"dpl": dpl, "oh": ohp})
    return in_maps


def _finish(results):
    """Host-side reduction of per-core partials -> scalar loss (float32).

    out1 [P, 4*129]: per class block, cols [:128] confusion block
    (trace = inter_c), col 128 = per-row tsum_c partials.
    """
    inter = np.zeros(NCLS, dtype=np.float64)
    tsum = np.zeros(NCLS, dtype=np.float64)
    for r in results:
        o = r["out1"].astype(np.float64)
        for ci in range(NCLS):
            blk = o[:, ci * 129 : (ci + 1) * 129]
            inter[ci] += np.trace(blk[:, :128])
            tsum[ci] += blk[:, 128].sum()
    inter = inter.astype(np.float32)
    tsum = tsum.astype(np.float32)
    eps = np.float32(EPS)
    dice = (np.float32(2.0) * inter + eps) / (inter + tsum + eps)
    loss = np.float32(1.0) - np.mean(dice, dtype=np.float32)
    return np.array([loss], dtype=np.float32)


# test.py can set e.g. RUN_KWARGS.update(trace=True) to profile; the grader
# path leaves this empty.
RUN_KWARGS = {}
LAST_RESULT = None


def kernel(input2, target1):
    global LAST_RESULT
    nc = build_program()
    in_maps = make_in_maps(input2, target1)
    res = run_bass_kernel_spmd(nc, in_maps, core_ids=list(range(N_CORES)), **RUN_KWARGS)
    LAST_RESULT = res
    return _finish(res.results)


# revision 30
# speedup vs baseline: 1.0570x; 1.0333x over previous
"""Dice-loss kernel for Trainium2 (Bass/Tile), 8-core data-parallel SPMD.

Strategy
--------
reference: pred = argmax_c(logits); for c in 1..4:
    inter_c = #{v : pred[v]==c and tgt[v]==c},  tsum_c = #{v : tgt[v]==c}
    dice_c = (2*inter_c + eps) / (inter_c + tsum_c + eps); loss = 1 - mean(dice)

The voxel axis (B*D*H*W = 7,077,888) is sharded 8 ways.  Host-side input
formatting (per-voxel, information-preserving maps only -- all 7M-voxel
reductions happen on device):
  - d_c = l_c - l0 (fp32 sub, fp16 store), c=1..4: argmax is per-voxel
    translation invariant, so pred==c iff d_c == max(d) and d_c >= 0.
  - one-hot labels t_c as fp8e4m3 planes (0.0/1.0 exact): feeds the PE
    confusion matmul directly.

Engine split (v4, balanced against the ~358 GB/s per-core HBM roofline):
  Sync:   ALL dma_starts (no compute waits ahead of DMA issue; the
          scalar-engine HWDGE ring caused head-of-line blocking when the
          per-tile relu sat in front of oh issues)
  Scalar: mz = relu(m') (the 0-clamp, off DVE) + final PSUM->SBUF copies
  DVE:    mab = pairwise max (TT @2x), m' = max(mab0, mab1) (TT @2x),
          ev = 4-plane is_ge vs broadcast mz (TT @2x)
  GpSimd: one-time ev ones-column memsets
  PE:     4 confusion matmuls per 128-chunk, 129 moving cols each: the
          ev layout interleaves a constant-1.0 column after every 128
          data cols, so PSUM col 128 accumulates colsums of the
          stationary one-hot = tsum_c FOR FREE (no separate ones-matmul;
          ones cols sit at col % 129 == 128 in every chunk layout, so a
          single strided memset per ev buffer survives all tiles).

All SBUF DMA dests are flat [p, 0:4*fd] so every transfer is one
contiguous run per partition (128 descriptors, 2-9KB packets).

Accuracy: fp16 d-plane ties give ~1.4e-4 relative error on the loss
(tolerance 2e-2).  Counts stay exact integers in fp32 accumulators.
"""

import sys
from contextlib import ExitStack

import numpy as np

for _p in ("/opt/trn_rl_repo", "/opt/pypackages"):
    if _p not in sys.path:
        sys.path.append(_p)

import ml_dtypes
import concourse.bacc as bacc
import concourse.bass as bass
import concourse.tile as tile
from concourse import mybir
from concourse.bass_utils import run_bass_kernel_spmd

# Problem shape (hardcoded per contract: kernel.py must be self-contained).
B, C, D, H, W = 2, 5, 96, 192, 192
N_CORES = 8
P = 128                      # SBUF partitions
NVOX = B * D * H * W         # 7,077,888 voxels
SHARD = NVOX // N_CORES      # 884,736 voxels per core
FTOT = SHARD // P            # 6,912 free elems per partition
TILES = [128, 256, 512, 896, 1280, 1280, 1280, 1024, 256]
NT = len(TILES)
NCLS = C - 1                 # foreground classes 1..4
EPS = 1e-8
assert sum(TILES) == FTOT


def emit_dice_kernel(tc, dpl_ap, oh_ap, out_ap, p, tiles):
    """Emit the per-core dice partial-sums program into TileContext `tc`.

    dpl_ap: DRAM [p, 4*ftot] fp16  -- d-planes, tile-blocked: cols
            [4*base, 4*(base+fd)) hold tile i as [4, fd] row-major
    oh_ap:  DRAM [p, 4*ftot] fp8e4 -- one-hot planes, same blocking
    out_ap: DRAM [p, 4*129] f32 -- per class c: cols [129c, 129c+128)
            confusion block (host takes the trace = inter_c), col
            129c+128 = per-row tsum_c partials.
    """
    nc = tc.nc
    nt = len(tiles)
    fdmax = max(tiles)
    ftot = sum(tiles)
    nchmax = fdmax // 128
    evw = 4 * nchmax * 129   # ev tile width (interleaved ones cols)
    fp16 = mybir.dt.float16
    fp8 = mybir.dt.float8e4
    f32 = mybir.dt.float32
    Alu = mybir.AluOpType
    Act = mybir.ActivationFunctionType
    assert all(fd % 128 == 0 for fd in tiles)

    with ExitStack() as ctx:
        pool_d = ctx.enter_context(tc.tile_pool(name="d", bufs=6))
        pool_oh = ctx.enter_context(tc.tile_pool(name="oh", bufs=6))
        pool_mab = ctx.enter_context(tc.tile_pool(name="mab", bufs=3))
        pool_mz = ctx.enter_context(tc.tile_pool(name="mz", bufs=4))
        pool_ev = ctx.enter_context(tc.tile_pool(name="ev", bufs=3))
        pool_acc = ctx.enter_context(tc.tile_pool(name="acc", bufs=1))
        pool_ps = ctx.enter_context(tc.tile_pool(name="ps", bufs=1, space="PSUM"))

        outb = pool_acc.tile([p, 4 * 129], f32, tag="outb")
        cm = [
            pool_ps.tile([128, 129], f32, tag=f"cm{q}", name=f"cm{q}")
            for q in range(4)
        ]

        # The 3 physical ev buffers rotate; cols == 128 (mod 129) are the
        # constant-1.0 columns in EVERY tile's chunk layout (class bases
        # are multiples of 129), so one strided memset each lasts forever.
        ev_bufs = [
            pool_ev.tile([p, evw], fp16, tag="ev", name=f"ev{q}")
            for q in range(3)
        ]
        for evb in ev_bufs:
            ones_ap = bass.AP(
                tensor=evb.tensor,
                offset=evb.offset + 128,
                ap=[list(evb.ap[0]), [129, 4 * nchmax]],
            )
            nc.gpsimd.memset(ones_ap, 1.0)

        bases = []
        b0 = 0
        for fd in tiles:
            bases.append(b0)
            b0 += fd
        dvs, ohs = {}, {}

        def issue_dv(i):
            fd = tiles[i]
            dv = pool_d.tile([p, 4 * fdmax], fp16, tag="dv")
            src_d = bass.AP(
                tensor=dpl_ap.tensor,
                offset=4 * bases[i],
                ap=[[4 * ftot, p], [1, 4 * fd]],
            )
            nc.sync.dma_start(out=dv[:, 0 : 4 * fd], in_=src_d)
            dvs[i] = dv

        def issue_oh(i):
            fd = tiles[i]
            oh = pool_oh.tile([p, 4 * fdmax], fp8, tag="oh")
            src_o = bass.AP(
                tensor=oh_ap.tensor,
                offset=4 * bases[i],
                ap=[[4 * ftot, p], [1, 4 * fd]],
            )
            nc.sync.dma_start(out=oh[:, 0 : 4 * fd], in_=src_o)
            ohs[i] = oh

        # pre-issue a few tiles' transfers so the DMA engines ramp before
        # the first compute dependency lands.  dpl first, oh staggered
        # two tiles behind: the 16 SDMA engines round-robin across live
        # queues at packet granularity, so every concurrently-issued
        # transfer delays dpl0's completion (and with it the first DVE
        # op).  Tiles 0-2 are small for the same reason.
        PRE_DV = 4
        PRE_OH = 1
        for i in range(PRE_DV):
            issue_dv(i)
        for i in range(PRE_OH):
            issue_oh(i)

        def emit_tail(i, fd, dv, oh, mz):
            """ev compare + confusion matmuls for tile i.  Emitted one
            iteration late (software pipelining): ev_i waits on the ACT
            relu; putting mab_{i+1}/m'_{i+1} ahead of it in the DVE queue
            hides that latency instead of head-of-line stalling."""
            nch = fd // 128
            # e_c = (d_c >= mz) for all 4 classes in ONE op, written into
            # the interleaved [128 data | 1 one] chunk layout (4D APs;
            # innermost packed 128 keeps the 2x DVE mode).
            ev = pool_ev.tile([p, evw], fp16, tag="ev")
            ev_v = bass.AP(
                tensor=ev.tensor,
                offset=ev.offset,
                ap=[list(ev.ap[0]), [129 * nch, 4], [129, nch], [1, 128]],
            )
            dv_v = bass.AP(
                tensor=dv.tensor,
                offset=dv.offset,
                ap=[list(dv.ap[0]), [fd, 4], [128, nch], [1, 128]],
            )
            m_sl = mz[:, 0:fd]
            m_bc = bass.AP(
                tensor=m_sl.tensor,
                offset=m_sl.offset,
                ap=[list(m_sl.ap[0]), [0, 4], [128, nch], [1, 128]],
            )
            nc.vector.tensor_tensor(ev_v, dv_v, m_bc, Alu.is_ge)

            # PE: per 128-chunk, 4 confusion matmuls (fp8 stationary x
            # fp16 moving, 129 cols: 128 ev data + the 1.0 col -> PSUM
            # col 128 accumulates colsums of oh_c = tsum_c partials).
            # On the last tile, run class-major with the per-class final
            # PSUM->SBUF copy right after that class's stop matmul, so
            # staging overlaps the remaining classes' matmuls.
            first = i == 0
            last = i == nt - 1
            if not last:
                for k in range(nch):
                    o = k * 128
                    st = first and k == 0
                    for ci in range(4):
                        nc.tensor.matmul(
                            cm[ci],
                            oh[:, ci * fd + o : ci * fd + o + 128],
                            ev[:, ci * (129 * nch) + k * 129 : ci * (129 * nch) + k * 129 + 129],
                            start=st,
                            stop=False,
                        )
            else:
                for ci in range(4):
                    for k in range(nch):
                        o = k * 128
                        nc.tensor.matmul(
                            cm[ci],
                            oh[:, ci * fd + o : ci * fd + o + 128],
                            ev[:, ci * (129 * nch) + k * 129 : ci * (129 * nch) + k * 129 + 129],
                            start=False,
                            stop=k == nch - 1,
                        )
                    nc.scalar.activation(
                        outb[:, ci * 129 : (ci + 1) * 129], cm[ci], Act.Copy
                    )
                    # ship this class's block immediately so the output
                    # DMA overlaps the remaining classes' matmuls
                    nc.sync.dma_start(
                        out=out_ap[:, ci * 129 : (ci + 1) * 129],
                        in_=outb[:, ci * 129 : (ci + 1) * 129],
                    )

        DIST = 2  # tail pipeline distance (tiles)
        pend = []
        for i, fd in enumerate(tiles):
            if i + PRE_DV < nt:
                issue_dv(i + PRE_DV)
            if i + PRE_OH < nt:
                issue_oh(i + PRE_OH)
            dv = dvs.pop(i)
            oh = ohs.pop(i)

            # mab = pairwise max of the 4 d-planes (2 planes out)
            mab = pool_mab.tile([p, 2 * fdmax], fp16, tag="mab")
            nc.vector.tensor_tensor(
                mab[:, 0 : 2 * fd],
                dv[:, 0 : 2 * fd],
                dv[:, 2 * fd : 4 * fd],
                Alu.max,
            )
            # m' = max(mab0, mab1)
            mp = pool_mz.tile([p, fdmax], fp16, tag="mp")
            nc.vector.tensor_tensor(
                mp[:, 0:fd], mab[:, 0:fd], mab[:, fd : 2 * fd], Alu.max
            )
            # mz = max(m', 0) -- tensor_scalar runs @4x on DVE; keeping
            # the whole chain on one engine avoids cross-engine stalls
            # the Tile scheduler cannot model (its ACT cost model misses
            # the TRN2 SBUF-source errata, so it pins ev right behind an
            # ACT relu and the DVE head-of-line stalls ~1.3us per tile).
            mz = pool_mz.tile([p, fdmax], fp16, tag="mz")
            nc.vector.tensor_scalar(mz[:, 0:fd], mp[:, 0:fd], 0.0, None, Alu.max)

            pend.append((i, fd, dv, oh, mz))
            if len(pend) > DIST:
                emit_tail(*pend.pop(0))
        for t in pend:
            emit_tail(*t)

        # PSUM staging + per-class output DMAs were interleaved with the
        # last tile's matmuls above; nothing left to emit.


_PROGRAM_CACHE = {}


def build_program():
    key = (C, P, FTOT, tuple(TILES))
    if key in _PROGRAM_CACHE:
        return _PROGRAM_CACHE[key]
    nc = bacc.Bacc("TRN2", debug=False, target_bir_lowering=False)
    dpl = nc.dram_tensor(
        "dpl", [P, 4 * FTOT], mybir.dt.float16, kind="ExternalInput"
    )
    oh = nc.dram_tensor(
        "oh", [P, 4 * FTOT], mybir.dt.float8e4, kind="ExternalInput"
    )
    out1 = nc.dram_tensor(
        "out1", [P, 4 * 129], mybir.dt.float32, kind="ExternalOutput"
    )
    with tile.TileContext(nc) as tc:
        emit_dice_kernel(tc, dpl.ap(), oh.ap(), out1.ap(), P, TILES)
    nc.compile()
    _PROGRAM_CACHE[key] = nc
    return nc


def make_in_maps(input2, target1):
    lg = np.asarray(input2, dtype=np.float32)
    tg = np.asarray(target1)
    # d_c = l_c - l0 in fp32, stored fp16; one-hot labels as fp8 (exact)
    d16 = (lg[:, 1:C] - lg[:, 0:1]).astype(np.float16).reshape(B, NCLS, NVOX // B)
    tgf = tg.reshape(B, NVOX // B)
    shards_per_b = N_CORES // B
    s = (NVOX // B) // shards_per_b
    in_maps = []
    for core in range(N_CORES):
        b, q = divmod(core, shards_per_b)
        sl = slice(q * s, (q + 1) * s)
        dsh = d16[b, :, sl].reshape(NCLS, P, FTOT)
        tsh = tgf[b, sl].reshape(P, FTOT)
        dpl = np.empty((P, 4 * FTOT), dtype=np.float16)
        ohp = np.empty((P, 4 * FTOT), dtype=ml_dtypes.float8_e4m3fn)
        base = 0
        for fd in TILES:
            slt = slice(base, base + fd)
            blk = slice(4 * base, 4 * (base + fd))
            dpl[:, blk] = dsh[:, :, slt].transpose(1, 0, 2).reshape(P, 4 * fd)
            ohc = np.stack(
                [(tsh[:, slt] == c) for c in range(1, C)], axis=1
            )  # [P, 4, fd] bool
            ohp[:, blk] = ohc.reshape(P, 4 * fd).astype(ml_dtypes.float8_e4m3fn)
            base += fd
        in_maps.append({"dpl": dpl, "oh": ohp})
    return in_maps


def _finish(results):
    """Host-side reduction of per-core partials -> scalar loss (float32).

    out1 [P, 4*129]: per class block, cols [:128] confusion block
    (trace = inter_c), col 128 = per-row tsum_c partials.
    """
    inter = np.zeros(NCLS, dtype=np.float64)
    tsum = np.zeros(NCLS, dtype=np.float64)
    for r in results:
        o = r["out1"].astype(np.float64)
        for ci in range(NCLS):
            blk = o[:, ci * 129 : (ci + 1) * 129]
            inter[ci] += np.trace(blk[:, :128])
            tsum[ci] += blk[:, 128].sum()
    inter = inter.astype(np.float32)
    tsum = tsum.astype(np.float32)
    eps = np.float32(EPS)
    dice = (np.float32(2.0) * inter + eps) / (inter + tsum + eps)
    loss = np.float32(1.0) - np.mean(dice, dtype=np.float32)
    return np.array([loss], dtype=np.float32)


# test.py can set e.g. RUN_KWARGS.update(trace=True) to profile; the grader
# path leaves this empty.
RUN_KWARGS = {}
LAST_RESULT = None


def kernel(input2, target1):
    global LAST_RESULT
    nc = build_program()
    in_maps = make_in_maps(input2, target1)
    res = run_bass_kernel_spmd(nc, in_maps, core_ids=list(range(N_CORES)), **RUN_KWARGS)
    LAST_RESULT = res
    return _finish(res.results)
